# revision 1
# baseline (speedup 1.0000x reference)
"""Trainium2 Bass kernel for nn_MhcModule_41798621724799.

Computation (per token s, D=4096 split into NS=4 streams of d=1024):
  rs        = rsqrt(mean(x^2) + eps)
  w         = (x * rs * gamma) @ phi^T                       [S, 24]
  h_pre     = sigmoid(w[:4]*a0 + b[:4]) (+eps)
  h_post    = 2*sigmoid(w[4:8]*a1 + b[4:8])
  h_res     = sinkhorn(softmax(w[8:24]*a2 + b[8:24]))        [S, 4, 4]
  M[s,j,i]  = h_post[s,i]*h_pre[s,j] + h_res[s,j,i]
  out[s,i,:]= sum_j M[s,j,i] * x[s,j,:]

Sharding: pure data parallel over tokens — 8 cores x 512 tokens.

Per-core structure (tokens on partitions, 4 tiles of 128 tokens):
  - PE transposes x blocks (f32r transpose-mode) -> bf16 xT for the
    w matmul (phi pre-transposed on chip with gamma/alpha folded in).
  - RMS via ACT Square+accumulate; rsqrt = exp(-0.5*ln(.)) so every ACT
    function (Square/Ln/Exp/Copy) lives in ONE table set (no reloads).
  - sigmoid(z) = 1/(1+exp(-z)) via Exp + fast reciprocal.
  - sinkhorn batched per pair of tiles (10 iters, eps dropped - converged).
  - mix out_i = sum_j M[j,i] x_j as PE matmuls with per-token diagonal
    weights (float32r: full speed at N=512).
"""

import sys

sys.path.insert(0, "/opt/trn_rl_repo")

import functools
from contextlib import ExitStack

import numpy as np

import concourse.bacc as bacc
import concourse.bass as bass
import concourse.tile as tile
from concourse import mybir
import concourse.hw_specs as hw_specs
from concourse.bass_utils import run_bass_kernel_spmd
from concourse.masks import make_identity

NS = 4
EPS = 1e-6
B, S, D = 1, 4096, 4096
N_CORES = 8
S_SHARD = S // N_CORES  # 512
P = 128
NT = S_SHARD // P  # 4 token tiles per core
NBLK = D // P  # 32 d-blocks
NK = 2 * NS + NS * NS  # 24
SINK_ITERS = 8  # sinkhorn normalization pairs (ref uses 20; converged by ~8-10)

f32 = mybir.dt.float32
f32r = mybir.dt.float32r
bf16 = mybir.dt.bfloat16
AF = mybir.ActivationFunctionType
OP = mybir.AluOpType
AX = mybir.AxisListType


# Keep Exp/Ln/Square only in the combined table set so the greedy set chooser
# never thrashes between exp_and_others and natural_log (1 load instead of 9).
@functools.cache
def _patched_act_tables(module_arch):
    tabs = hw_specs.get_activation_tables(module_arch)
    combined = "natural_log_exp_and_others"
    if combined in tabs:
        special = {AF.Exp, AF.Ln, AF.Square}
        tabs = {
            k: (set(v) if k == combined else set(v) - special)
            for k, v in tabs.items()
        }
    return tabs


bacc.get_activation_tables = _patched_act_tables


def _ap(ap, dims):
    """Build an AP over the same tensor with explicit [step, count] dims."""
    return bass.AP(tensor=ap.tensor, offset=ap.offset, ap=dims)


def _mhc_body(ctx, tc, x, phi, alpha, beta, gamma, out, reps=1):
    nc = tc.nc

    consts = ctx.enter_context(tc.tile_pool(name="consts", bufs=1))
    xpool = ctx.enter_context(tc.tile_pool(name="xpool", bufs=NT))
    sqpool = ctx.enter_context(tc.tile_pool(name="sqpool", bufs=1))
    xtpool = ctx.enter_context(tc.tile_pool(name="xtpool", bufs=2))
    xbfpool = ctx.enter_context(tc.tile_pool(name="xbfpool", bufs=2))
    smalls = ctx.enter_context(tc.tile_pool(name="smalls", bufs=2 * NT))
    hpool = ctx.enter_context(tc.tile_pool(name="hpool", bufs=2))
    mpool = ctx.enter_context(tc.tile_pool(name="mpool", bufs=2))
    diagpool = ctx.enter_context(tc.tile_pool(name="diagpool", bufs=2))
    outpool = ctx.enter_context(tc.tile_pool(name="outpool", bufs=2))
    sink = ctx.enter_context(tc.tile_pool(name="sink", bufs=4))

    # banks: psum_t 3 + psum_s 1 + psum_mix 2x2 = 8. psum_s at 1 buf is ~free:
    # its consumers (wps -> w_sb -> wtp) are data-dependent anyway; the extra
    # psum_t buf lets the PE transpose stream run 3 groups ahead of evacuation.
    psum_t = ctx.enter_context(tc.tile_pool(name="psum_t", bufs=3, space="PSUM"))
    psum_s = ctx.enter_context(tc.tile_pool(name="psum_s", bufs=1, space="PSUM"))
    psum_mix = ctx.enter_context(tc.tile_pool(name="psum_mix", bufs=2, space="PSUM"))

    # ---------------- constants ----------------
    ident = consts.tile([P, P], f32)
    make_identity(nc, ident)
    ident_r = consts.tile([P, P], f32r)
    nc.vector.tensor_copy(out=ident_r, in_=ident)
    ident_bf = consts.tile([P, P], bf16)
    nc.vector.tensor_copy(out=ident_bf, in_=ident)

    # phi padded to 128 partitions so PE transposes use the full array
    # (walrus codegen chokes on tile_position'd LDWEIGHTS with multiple waits)
    phi_sb = consts.tile([P, D], f32)
    nc.vector.memset(phi_sb[:], 0.0)
    nc.sync.dma_start(out=phi_sb[:NK, :], in_=phi)
    gamma_sb = consts.tile([P, NBLK], f32)
    nc.gpsimd.dma_start(out=gamma_sb, in_=gamma.rearrange("(b p) -> p b", p=P))
    alpha_bc = consts.tile([P, 3], f32)
    nc.gpsimd.dma_start(out=alpha_bc, in_=_ap(alpha, [[0, P], alpha.ap[0]]))
    beta_bc = consts.tile([P, NK], f32)
    nc.gpsimd.dma_start(out=beta_bc, in_=_ap(beta, [[0, P], beta.ap[0]]))
    eps_t = consts.tile([P, 1], f32)
    nc.vector.memset(eps_t, EPS)

    # phiT[p, b, k] = phi[k, b*128+p] * gamma[b*128+p] * alpha[branch(k)], bf16
    phiT_f = consts.tile([P, NBLK, NK], f32)
    phiT = consts.tile([P, NBLK, NK], bf16)
    for b in range(NBLK):
        pt = psum_s.tile([P, P], f32, tag="psum_small")
        nc.tensor.transpose(pt, phi_sb[:, b * P : (b + 1) * P], ident)
        nc.vector.tensor_scalar_mul(
            out=phiT_f[:, b, :], in0=pt[:, :NK], scalar1=gamma_sb[:, b : b + 1]
        )
    for j, (k0, k1) in enumerate([(0, NS), (NS, 2 * NS), (2 * NS, NK)]):
        nc.vector.tensor_scalar_mul(
            out=phiT[:, :, k0:k1], in0=phiT_f[:, :, k0:k1], scalar1=alpha_bc[:, j : j + 1]
        )

    for _rep in range(reps):
        _mhc_rep(tc, x, out, xpool, sqpool, xtpool, xbfpool, smalls, hpool,
                 mpool, diagpool, outpool, sink, psum_t, psum_s, psum_mix,
                 ident, ident_r, ident_bf, beta_bc, phiT, eps_t)


def _sinkhorn(nc, h, nt, iters):
    """In-place sinkhorn on h [P, nt, NS(j), NS(i)], no eps (converged)."""
    sink = _sinkhorn.pool
    for _ in range(iters):
        rsum = sink.tile([P, nt, NS], f32, tag="rsum", bufs=2)
        nc.vector.tensor_reduce(out=rsum, in_=h[:], axis=AX.X, op=OP.add)
        rrec = sink.tile([P, nt, NS], f32, tag="rrec", bufs=2)
        nc.vector.reciprocal_approx_fast(out=rrec, in_=rsum)
        ra = rrec[:]
        rb = _ap(ra, [ra.ap[0], ra.ap[1], ra.ap[2], [0, NS]])
        nc.vector.tensor_mul(out=h[:], in0=h[:], in1=rb)

        csum = sink.tile([P, nt, NS], f32, tag="csum", bufs=2)
        nc.vector.tensor_reduce(
            out=csum, in_=h[:].rearrange("p t j i -> p t i j"), axis=AX.X, op=OP.add
        )
        crec = sink.tile([P, nt, NS], f32, tag="crec", bufs=2)
        nc.vector.reciprocal_approx_fast(out=crec, in_=csum)
        ca = crec[:]
        cb = _ap(ca, [ca.ap[0], ca.ap[1], [0, NS], ca.ap[2]])
        nc.vector.tensor_mul(out=h[:], in0=h[:], in1=cb)


def _mhc_rep(tc, x, out, xpool, sqpool, xtpool, xbfpool, smalls, hpool,
             mpool, diagpool, outpool, sink, psum_t, psum_s, psum_mix,
             ident, ident_r, ident_bf, beta_bc, phiT, eps_t):
    nc = tc.nc
    _sinkhorn.pool = sink
    x_tiles = []
    h_pairs = []
    h01_tiles = []
    # ---------------- phase A + per-pair sinkhorn ----------------
    for pair in range(NT // 2):
        xt = xtpool.tile([P, NBLK, 2 * P], bf16, tag="xt")
        h_pair = hpool.tile([P, 2, NS, NS], f32, tag="h_pair")
        h_pairs.append(h_pair)
        rs_tiles = []
        for tl in range(2):
            t = pair * 2 + tl
            # x tile is float32r-typed so the f32r matmuls accept it;
            # non-matmul consumers read it through a f32 bitcast view.
            x_t = xpool.tile([P, D], f32r, tag="x_t")
            nc.sync.dma_start(out=x_t, in_=x[t * P : (t + 1) * P, :].bitcast(f32r))
            x_tiles.append(x_t)
            x_t_f32 = x_t[:].bitcast(f32)

            # RMS: ssq = sum(x^2); rs = exp(-0.5*ln(ssq/D + eps)).
            # The activation's main output is a dummy — write it through a
            # 0-stride AP so it costs no SBUF.
            sqs = sqpool.tile([P, 1], f32, tag="sqs")
            ssq = smalls.tile([P, 1], f32, tag="ssq")
            sq_ap = sqs[:]
            nc.scalar.activation(
                out=_ap(sq_ap, [sq_ap.ap[0], [0, D]]),
                in_=x_t_f32, func=AF.Square, accum_out=ssq,
            )
            lnm = smalls.tile([P, 1], f32, tag="lnm")
            nc.scalar.activation(
                out=lnm, in_=ssq, func=AF.Ln, scale=1.0 / D, bias=eps_t[:]
            )
            rs = smalls.tile([P, 1], f32, tag="rs")
            nc.scalar.activation(out=rs, in_=lnm, func=AF.Exp, scale=-0.5)
            rs_tiles.append(rs)

            # cast x to bf16 first: bf16 transpose-mode is 2x faster than f32
            # (1 vs 2 cyc/row) and bf16 PSUM reads evacuate at 2x DVE mode.
            # Precision is identical — xT was bf16 downstream anyway.
            xbf = xbfpool.tile([P, D], bf16, tag="xbf")
            nc.vector.tensor_copy(out=xbf[:, : D // 2], in_=x_t_f32[:, : D // 2])
            nc.scalar.copy(out=xbf[:, D // 2 :], in_=x_t_f32[:, D // 2 :])

            # transpose x into xT, 8 blocks per psum bank (bf16).
            # The leading throwaway matmuls per group are real (non-transpose)
            # PE activity: transpose-mode alone never un-gates the HAM clock,
            # so without them the whole transpose stream runs at 1.2 GHz.
            for g in range(4):
                pt = psum_t.tile([P, 8, P], bf16, tag="psum_t")
                # warm-up matmuls write f32 into bytes the first transposes
                # overwrite right after (start=True clears the written range)
                nc.tensor.matmul(
                    pt[:, 0, :].bitcast(f32), lhsT=ident_r, rhs=ident_r[:, :64],
                    start=True, stop=True,
                )
                nc.tensor.matmul(
                    pt[:, 1, :].bitcast(f32), lhsT=ident_r, rhs=ident_r[:, :64],
                    start=True, stop=True,
                )
                for b8 in range(8):
                    b = g * 8 + b8
                    nc.tensor.transpose(
                        pt[:, b8, :], xbf[:, b * P : (b + 1) * P], ident_bf
                    )
                dst = xt[:, g * 8 : (g + 1) * 8, tl * P : (tl + 1) * P]
                if g % 2 == 0:
                    nc.vector.tensor_copy(out=dst, in_=pt)
                else:
                    nc.scalar.copy(out=dst, in_=pt)

        # w^T = phiT^T @ xT : [24, 256] accumulated over 32 d-blocks
        wps = psum_s.tile([NK, 2 * P], f32, tag="psum_small")
        for b in range(NBLK):
            nc.tensor.matmul(
                wps,
                lhsT=phiT[:, b, :],
                rhs=xt[:, b, :],
                start=(b == 0),
                stop=(b == NBLK - 1),
            )
        w_sb = smalls.tile([P, 2 * P], f32, tag="w_sb")
        nc.vector.memset(w_sb[:], 0.0)
        nc.vector.tensor_copy(out=w_sb[:NK, :], in_=wps)

        for tl in range(2):
            t = pair * 2 + tl
            wtp = psum_s.tile([P, P], f32, tag="psum_small")
            nc.tensor.transpose(wtp, w_sb[:, tl * P : (tl + 1) * P], ident)
            # z = w*rs + beta   (alpha/gamma already folded into phiT)
            z = smalls.tile([P, NK], f32, tag="z")
            nc.vector.scalar_tensor_tensor(
                out=z, in0=wtp[:, :NK], scalar=rs_tiles[tl], in1=beta_bc,
                op0=OP.mult, op1=OP.add,
            )
            # sigmoids for h_pre (k 0:4) and h_post/2 (k 4:8), via exp
            ez = smalls.tile([P, 2 * NS], f32, tag="ez")
            nc.scalar.activation(out=ez, in_=z[:, 0 : 2 * NS], func=AF.Exp, scale=-1.0)
            ez1 = smalls.tile([P, 2 * NS], f32, tag="ez1")
            nc.vector.tensor_scalar_add(out=ez1, in0=ez, scalar1=1.0)
            h01 = smalls.tile([P, 2 * NS], f32, tag="h01")
            nc.vector.reciprocal_approx_fast(out=h01, in_=ez1)
            h01_tiles.append(h01)
            # h_res seed: exp(z)  (unnormalized; sinkhorn row-norm handles it)
            nc.scalar.activation(
                out=h_pair[:, tl].rearrange("p j i -> p (j i)"),
                in_=z[:, 2 * NS : NK],
                func=AF.Exp,
            )

        _sinkhorn(nc, h_pair, 2, SINK_ITERS)

    # ---------------- phase C: mixing matrices + output ----------------
    for t in range(NT):
        M = mpool.tile([P, NS * NS], f32, tag="M")
        M3 = M[:].rearrange("p (j i) -> p j i", i=NS)
        h01 = h01_tiles[t]
        hp = h01[:, 0:NS]
        hq = h01[:, NS : 2 * NS]
        hp_b = _ap(hp, [hp.ap[0], hp.ap[1], [0, NS]])  # [p, j, i-bcast]
        hq_b = _ap(hq, [hq.ap[0], [0, NS], hq.ap[1]])  # [p, j-bcast, i]
        # M = (h_pre * 2) * sigmoid_post + h_res
        nc.vector.scalar_tensor_tensor(
            out=M3, in0=hp_b, scalar=2.0, in1=hq_b, op0=OP.mult, op1=OP.mult
        )
        nc.vector.tensor_add(out=M3, in0=M3, in1=h_pairs[t // 2][:, t % 2])

        diag = diagpool.tile([P, NS * NS, P], f32r, tag="diag")
        for ji in range(NS * NS):
            nc.vector.tensor_scalar_mul(
                out=diag[:, ji, :], in0=ident, scalar1=M[:, ji : ji + 1]
            )

        out_sb = outpool.tile([P, NS, 2, 512], f32, tag="out_sb")
        x_t = x_tiles[t]
        for i in range(NS):
            ps = psum_mix.tile([P, 2, 512], f32, tag="psum_mix")
            for j in range(NS):
                c0 = j * 1024
                nc.tensor.matmul(
                    ps[:, 0, :],
                    lhsT=diag[:, j * NS + i, :],
                    rhs=x_t[:, c0 : c0 + 512],
                    start=(j == 0),
                    stop=(j == NS - 1),
                )
            for j in range(NS):
                c0 = j * 1024 + 512
                nc.tensor.matmul(
                    ps[:, 1, :],
                    lhsT=diag[:, j * NS + i, :],
                    rhs=x_t[:, c0 : c0 + 512],
                    start=(j == 0),
                    stop=(j == NS - 1),
                )
            if i % 2 == 0:
                nc.vector.tensor_copy(out=out_sb[:, i], in_=ps)
            else:
                nc.scalar.copy(out=out_sb[:, i], in_=ps)
                # store each half-tile as soon as its two chunks are
                # evacuated (ACT HWDGE ring, away from the x loads) —
                # overlaps stores with the remaining mix matmuls
                nc.scalar.dma_start(
                    out=out[t * P : (t + 1) * P, (i - 1) * 1024 : (i + 1) * 1024],
                    in_=out_sb[:, i - 1 : i + 1],
                )


def build_bass(reps=1):
    nc = bacc.Bacc("TRN2", target_bir_lowering=False, debug=False)
    x = nc.dram_tensor("x", [S_SHARD, D], f32, kind="ExternalInput").ap()
    phi = nc.dram_tensor("phi", [NK, D], f32, kind="ExternalInput").ap()
    alpha = nc.dram_tensor("alpha", [3], f32, kind="ExternalInput").ap()
    beta = nc.dram_tensor("beta", [NK], f32, kind="ExternalInput").ap()
    gamma = nc.dram_tensor("gamma", [D], f32, kind="ExternalInput").ap()
    out = nc.dram_tensor("out", [S_SHARD, D], f32, kind="ExternalOutput").ap()
    with tile.TileContext(nc) as tc:
        with ExitStack() as ctx:
            _mhc_body(ctx, tc, x, phi, alpha, beta, gamma, out, reps=reps)
    nc.compile()
    return nc


_NC_CACHE = {}


def _get_nc():
    if "nc" not in _NC_CACHE:
        _NC_CACHE["nc"] = build_bass()
    return _NC_CACHE["nc"]


def make_in_maps(x, phi_weight, branch_alpha, branch_beta, norm_gamma):
    xs = np.ascontiguousarray(np.asarray(x), dtype=np.float32).reshape(S, D)
    phi = np.ascontiguousarray(np.asarray(phi_weight), dtype=np.float32)
    al = np.ascontiguousarray(np.asarray(branch_alpha), dtype=np.float32)
    be = np.ascontiguousarray(np.asarray(branch_beta), dtype=np.float32)
    ga = np.ascontiguousarray(np.asarray(norm_gamma), dtype=np.float32)
    in_maps = []
    for c in range(N_CORES):
        in_maps.append(
            {
                "x": np.ascontiguousarray(xs[c * S_SHARD : (c + 1) * S_SHARD]),
                "phi": phi,
                "alpha": al,
                "beta": be,
                "gamma": ga,
            }
        )
    return in_maps


def kernel(x, phi_weight, branch_alpha, branch_beta, norm_gamma, _trace=False):
    nc = _get_nc()
    in_maps = make_in_maps(x, phi_weight, branch_alpha, branch_beta, norm_gamma)
    res = run_bass_kernel_spmd(
        nc, in_maps, core_ids=list(range(N_CORES)), trace=_trace
    )
    out = np.concatenate([r["out"] for r in res.results], axis=0)
    if _trace:
        kernel.last_results = res
    return out.reshape(B, S, D).astype(np.float32)

